# revision 20
# baseline (speedup 1.0000x reference)
"""EvoAttentionCausal Trainium2 kernel (8-core SPMD).

Computes, per (b, h):
    v_swiglu = silu(V) * V
    c        = cumsum(v_swiglu, axis=S)
    c_n      = c * rsqrt(mean(c^2, D) + 1e-5)
    r        = ||Q||_D + ||K||_D + 1          (L2 eps 1e-8 is negligible)
    mstate   = c_n / r
    out0     = V + silu(mstate) * V
    out      = out0 * rsqrt(mean(out0^2, D) + 1e-5)

Sharding: B*H = 64 (b,h) units, 8 per NeuronCore, fully independent.

Layout: sequence on partitions ([128 s, 64 d] tiles, 32 tiles per bh).
Cumsum over S via tensor-engine matmuls:
  - per-tile prefix:   c_t = L128 @ w_t          (lhsT = upper-tri ones)
  - per-tile colsums:  colsums[t] = E_t^T @ w_t  (accumulated in one PSUM tile)
  - exclusive prefix of colsums: carries = Lx32 @ colsums
  - carry broadcast:   c_t += ones_col @ carries[t]  (PSUM accumulate)
All rsqrt/sqrt on DVE (Newton + bit trick) so ACT stays on the
silu_and_others table set (Silu/Square/Copy/Identity) -> one table load.
"""

import sys

sys.path.insert(0, "/opt/trn_rl_repo")

import numpy as np

import concourse.bass as bass
import concourse.bacc as bacc
import concourse.mybir as mybir
import concourse.tile as tile
from concourse.bass_utils import run_bass_kernel_spmd

F32 = mybir.dt.float32
U32 = mybir.dt.uint32
AF = mybir.ActivationFunctionType
OP = mybir.AluOpType
AX = mybir.AxisListType

B, H, S, D = 4, 16, 4096, 64
NCORES = 8
BH = B * H              # 64
BHC = BH // NCORES      # 8 bh units per core
P = 128                 # partitions
T = S // P              # 32 tiles per bh
GRP = 2                 # bh per row-math group (SBUF pressure)
NG = BHC // GRP         # 4 groups
NG1 = int(__import__("os").environ.get("NG1", NG))
RMS_EPS = 1e-5

_MAGIC = 0x5F3759DF


def _newton_rsqrt(nc, pool, x, magic, shift1, n, tag):
    """rsqrt(x) elementwise on a [128, n] f32 SBUF tile via bit trick +
    2 Newton iterations. Returns a fresh tile."""
    y = pool.tile([P, n], F32, tag=f"nwt_y_{tag}")
    t1 = pool.tile([P, n], F32, tag=f"nwt_t_{tag}")
    xh = pool.tile([P, n], F32, tag=f"nwt_xh_{tag}")
    # y0 bits = magic - (bits(x) >> 1)
    nc.vector.tensor_tensor(
        t1.bitcast(U32), x.bitcast(U32), shift1[:, 0:n], op=OP.logical_shift_right
    )
    nc.vector.tensor_tensor(
        y.bitcast(U32), magic[:, 0:n], t1.bitcast(U32), op=OP.subtract
    )
    nc.vector.tensor_scalar(xh, x, 0.5, None, op0=OP.mult)
    for _ in range(2):
        # y = y * (1.5 - xh * y * y)
        nc.vector.tensor_tensor(t1, y, y, op=OP.mult)
        nc.vector.tensor_tensor(t1, t1, xh, op=OP.mult)
        nc.vector.tensor_scalar(t1, t1, -1.0, 1.5, op0=OP.mult, op1=OP.add)
        nc.vector.tensor_tensor(y, y, t1, op=OP.mult)
    return y


def build_consts():
    """Host-side constant matrices shipped as extra kernel inputs."""
    # lhsT for c_t = L @ w_t  => upper-tri incl diag
    ut = np.triu(np.ones((P, P), np.float32))
    # sliding-window selector: column 31 is ones; E_t = slide[:, 31-t : 63-t]
    slide = np.zeros((P, 2 * T - 1), np.float32)
    slide[:, T - 1] = 1.0
    # lhsT for carries = Lx32 @ colsums => strictly upper tri [32, 32]
    sxt = np.triu(np.ones((T, T), np.float32), 1)
    # carry-broadcast selectors: esel[t] is [32, 128] with row t all ones,
    # so esel[t].T @ carries = ones(128) (x) carries[t]
    esel = np.zeros((T, T, P), np.float32)
    for t in range(T):
        esel[t, t, :] = 1.0
    esel = esel.transpose(1, 0, 2).reshape(T, T * P)
    magic = np.full((P, GRP * T), _MAGIC, np.uint32)
    shift1 = np.full((P, GRP * T), 1, np.uint32)
    return {
        "c_ut": ut, "c_slide": slide, "c_sxt": sxt, "c_esel": esel,
        "c_magic": magic, "c_shift1": shift1,
    }


def build_nc():
    nc = bacc.Bacc(
        "TRN2", target_bir_lowering=False, debug=False, num_devices=1
    )
    qd = nc.declare_dram_parameter("Q", [BHC, S, D], F32, isOutput=False)
    kd = nc.declare_dram_parameter("K", [BHC, S, D], F32, isOutput=False)
    vd = nc.declare_dram_parameter("V", [BHC, S, D], F32, isOutput=False)
    utd = nc.declare_dram_parameter("c_ut", [P, P], F32, isOutput=False)
    slided = nc.declare_dram_parameter("c_slide", [P, 2 * T - 1], F32, isOutput=False)
    sxtd = nc.declare_dram_parameter("c_sxt", [T, T], F32, isOutput=False)
    eseld = nc.declare_dram_parameter("c_esel", [T, T * P], F32, isOutput=False)
    magicd = nc.declare_dram_parameter("c_magic", [P, GRP * T], U32, isOutput=False)
    shiftd = nc.declare_dram_parameter("c_shift1", [P, GRP * T], U32, isOutput=False)
    outd = nc.declare_dram_parameter("out", [BHC, S, D], F32, isOutput=True)

    with tile.TileContext(nc) as tc:
        with (
            tc.tile_pool(name="consts", bufs=1) as consts,
            tc.tile_pool(name="vpool", bufs=GRP + 1) as vpool,
            tc.tile_pool(name="cpool", bufs=GRP) as cpool,
            tc.tile_pool(name="opool", bufs=GRP) as opool,
            tc.tile_pool(name="scr", bufs=2) as scr,
            tc.tile_pool(name="small", bufs=2) as small,
            tc.tile_pool(name="rowm", bufs=1) as rowm,
            tc.tile_pool(name="pc", bufs=2, space="PSUM") as ppc,
            tc.tile_pool(name="pcs", bufs=2, space="PSUM") as ppcs,
            tc.tile_pool(name="pcar", bufs=2, space="PSUM") as ppcar,
        ):
            ut = consts.tile([P, P], F32)
            nc.sync.dma_start(out=ut, in_=utd[:, :])
            slide = consts.tile([P, 2 * T - 1], F32)
            nc.sync.dma_start(out=slide, in_=slided[:, :])
            sxt = consts.tile([T, T], F32)
            nc.sync.dma_start(out=sxt, in_=sxtd[:, :])
            esel = consts.tile([T, T * P], F32)
            nc.sync.dma_start(out=esel, in_=eseld[:, :])
            magic = consts.tile([P, GRP * T], U32)
            nc.sync.dma_start(out=magic, in_=magicd[:, :])
            shift1 = consts.tile([P, GRP * T], U32)
            nc.sync.dma_start(out=shift1, in_=shiftd[:, :])

            for g in range(NG1):
                bhs = [g * GRP + i for i in range(GRP)]
                n = GRP * T
                # group stat tiles: [128, GRP*32]
                ccg = rowm.tile([P, n], F32, tag="ccg")
                qqg = rowm.tile([P, n], F32, tag="qqg")
                kkg = rowm.tile([P, n], F32, tag="kkg")

                vts, cts = {}, {}
                for i, bh in enumerate(bhs):
                    vt = vpool.tile([P, T, D], F32, tag="v")
                    nc.sync.dma_start(
                        out=vt, in_=vd[bh].rearrange("(t p) d -> p t d", p=P)
                    )
                    vts[bh] = vt
                    # silu(V) and w = silu(V)*V
                    sv = scr.tile([P, T, D], F32, tag="sv")
                    nc.scalar.activation(sv, vt, AF.Silu)
                    wt = scr.tile([P, T, D], F32, tag="w")
                    nc.vector.tensor_tensor(wt, sv, vt, op=OP.mult)

                    # Q/K norms^2 -> qqg/kkg
                    for name, dram, dst in (("q", qd, qqg), ("k", kd, kkg)):
                        xt = scr.tile([P, T, D], F32, tag="qk")
                        nc.sync.dma_start(
                            out=xt, in_=dram[bh].rearrange("(t p) d -> p t d", p=P)
                        )
                        sq = scr.tile([P, T, D], F32, tag="sq")
                        nc.gpsimd.tensor_tensor(sq, xt, xt, op=OP.mult)
                        nc.vector.tensor_reduce(
                            dst[:, i * T:(i + 1) * T], sq, axis=AX.X, op=OP.add
                        )

                    # colsums[t, :] = sum_p w_t[p, :]
                    pcs = ppcs.tile([T, D], F32)
                    for t in range(T):
                        nc.tensor.matmul(
                            pcs, slide[:, T - 1 - t:2 * T - 1 - t], wt[:, t, :],
                            start=(t == 0), stop=(t == T - 1),
                        )
                    cs_s = small.tile([T, D], F32, tag="cs")
                    nc.scalar.copy(cs_s, pcs)
                    # carries = exclusive prefix of colsums
                    pcar = ppcar.tile([T, D], F32)
                    nc.tensor.matmul(pcar, sxt, cs_s, start=True, stop=True)
                    car_s = small.tile([T, D], F32, tag="car")
                    nc.scalar.copy(car_s, pcar)

                    # c = L @ w_t + bcast(carries[t]), in 2 PSUM chunks
                    ct = cpool.tile([P, T, D], F32, tag="c")
                    cts[bh] = ct
                    for half in range(2):
                        pc = ppc.tile([P, T // 2, D], F32)
                        for tt in range(T // 2):
                            t = half * (T // 2) + tt
                            nc.tensor.matmul(
                                pc[:, tt, :], ut, wt[:, t, :],
                                start=True, stop=False,
                            )
                            nc.tensor.matmul(
                                pc[:, tt, :], esel[:, t * P:(t + 1) * P], car_s,
                                start=False, stop=True,
                            )
                        nc.scalar.copy(ct[:, half * (T // 2):(half + 1) * (T // 2), :], pc)

                    # cc = sum_d c^2
                    sqc = scr.tile([P, T, D], F32, tag="sq")
                    nc.scalar.square(sqc, ct)
                    nc.vector.tensor_reduce(
                        ccg[:, i * T:(i + 1) * T], sqc, axis=AX.X, op=OP.add
                    )

                # --- group row math: s1 = rsqrt((cc/64+eps) * r^2) ---
                rq = _newton_rsqrt(nc, rowm, qqg, magic, shift1, n, "q")
                nc.vector.tensor_tensor(rq, rq, qqg, op=OP.mult)  # sqrt(qq)
                rk = _newton_rsqrt(nc, rowm, kkg, magic, shift1, n, "k")
                nc.vector.tensor_tensor(rk, rk, kkg, op=OP.mult)  # sqrt(kk)
                r = rowm.tile([P, n], F32, tag="r")
                nc.vector.scalar_tensor_tensor(
                    r, rq, 1.0, rk, op0=OP.add, op1=OP.add
                )
                ccp = rowm.tile([P, n], F32, tag="ccp")
                nc.vector.tensor_scalar(
                    ccp, ccg, 1.0 / D, RMS_EPS, op0=OP.mult, op1=OP.add
                )
                nc.vector.tensor_tensor(ccp, ccp, r, op=OP.mult)
                nc.vector.tensor_tensor(ccp, ccp, r, op=OP.mult)
                s1 = _newton_rsqrt(nc, rowm, ccp, magic, shift1, n, "s1")

                oog = rowm.tile([P, n], F32, tag="oog")
                o0s = {}
                for i, bh in enumerate(bhs):
                    ct, vt = cts[bh], vts[bh]
                    ms = scr.tile([P, T, D], F32, tag="sv")
                    for t in range(T):
                        nc.vector.tensor_scalar(
                            ms[:, t, :], ct[:, t, :],
                            s1[:, i * T + t:i * T + t + 1], None, op0=OP.mult,
                        )
                    gt = scr.tile([P, T, D], F32, tag="w")
                    nc.scalar.activation(gt, ms, AF.Silu)
                    o0 = opool.tile([P, T, D], F32, tag="o0")
                    nc.vector.scalar_tensor_tensor(
                        o0, gt, 1.0, vt, op0=OP.add, op1=OP.mult
                    )
                    o0s[bh] = o0
                    sqo = scr.tile([P, T, D], F32, tag="sq")
                    nc.scalar.square(sqo, o0)
                    nc.vector.tensor_reduce(
                        oog[:, i * T:(i + 1) * T], sqo, axis=AX.X, op=OP.add
                    )

                oop = rowm.tile([P, n], F32, tag="ccp")
                nc.vector.tensor_scalar(
                    oop, oog, 1.0 / D, RMS_EPS, op0=OP.mult, op1=OP.add
                )
                s2 = _newton_rsqrt(nc, rowm, oop, magic, shift1, n, "s2")

                for i, bh in enumerate(bhs):
                    o0 = o0s[bh]
                    ot = cpool.tile([P, T, D], F32, tag="c")
                    for t in range(T):
                        nc.vector.tensor_scalar(
                            ot[:, t, :], o0[:, t, :],
                            s2[:, i * T + t:i * T + t + 1], None, op0=OP.mult,
                        )
                    nc.sync.dma_start(
                        out=outd[bh].rearrange("(t p) d -> p t d", p=P), in_=ot
                    )
    nc.finalize()
    return nc


_NC_CACHE = None


def kernel(Q, K, V):
    global _NC_CACHE
    if _NC_CACHE is None:
        _NC_CACHE = build_nc()
    nc = _NC_CACHE
    consts = build_consts()
    Qs = np.ascontiguousarray(np.asarray(Q, np.float32).reshape(BH, S, D))
    Ks = np.ascontiguousarray(np.asarray(K, np.float32).reshape(BH, S, D))
    Vs = np.ascontiguousarray(np.asarray(V, np.float32).reshape(BH, S, D))
    in_maps = []
    for c in range(NCORES):
        sl = slice(c * BHC, (c + 1) * BHC)
        in_maps.append(
            {"Q": Qs[sl], "K": Ks[sl], "V": Vs[sl], **consts}
        )
    res = run_bass_kernel_spmd(nc, in_maps, list(range(NCORES)))
    out = np.concatenate([res.results[c]["out"] for c in range(NCORES)], axis=0)
    return out.reshape(B, H, S, D)


# revision 24
# speedup vs baseline: 1.2697x; 1.2697x over previous
"""EvoAttentionCausal Trainium2 kernel (8-core SPMD).

Computes, per (b, h):
    v_swiglu = silu(V) * V
    c        = cumsum(v_swiglu, axis=S)
    c_n      = c * rsqrt(mean(c^2, D) + 1e-5)
    r        = ||Q||_D + ||K||_D + 1          (L2 eps 1e-8 is negligible)
    mstate   = c_n / r
    out0     = V + silu(mstate) * V
    out      = out0 * rsqrt(mean(out0^2, D) + 1e-5)

Sharding: B*H = 64 (b,h) units, 8 per NeuronCore, fully independent.

Layout: sequence on partitions ([128 s, 64 d] tiles, 32 tiles per bh).
Cumsum over S via tensor-engine matmuls:
  - per-tile prefix:   c_t = L128 @ w_t          (lhsT = upper-tri ones)
  - per-tile colsums:  colsums[t] = E_t^T @ w_t  (accumulated in one PSUM tile)
  - exclusive prefix of colsums: carries = Lx32 @ colsums
  - carry broadcast:   c_t += ones_col @ carries[t]  (PSUM accumulate)
All rsqrt/sqrt on DVE (Newton + bit trick) so ACT stays on the
silu_and_others table set (Silu/Square/Copy/Identity) -> one table load.
"""

import sys

sys.path.insert(0, "/opt/trn_rl_repo")

import numpy as np

import concourse.bass as bass
import concourse.bacc as bacc
import concourse.mybir as mybir
import concourse.tile as tile
from concourse.bass_utils import run_bass_kernel_spmd

F32 = mybir.dt.float32
U32 = mybir.dt.uint32
AF = mybir.ActivationFunctionType
OP = mybir.AluOpType
AX = mybir.AxisListType

B, H, S, D = 4, 16, 4096, 64
NCORES = 8
BH = B * H              # 64
BHC = BH // NCORES      # 8 bh units per core
P = 128                 # partitions
T = S // P              # 32 tiles per bh
GRP = 2                 # bh per row-math group (SBUF pressure)
NG = BHC // GRP         # 4 groups
NG1 = int(__import__("os").environ.get("NG1", NG))
RMS_EPS = 1e-5

_MAGIC = 0x5F3759DF


def _newton_rsqrt(nc, pool, x, magic, shift1, n, tag):
    """rsqrt(x) elementwise on a [128, n] f32 SBUF tile via bit trick +
    2 Newton iterations. Returns a fresh tile."""
    y = pool.tile([P, n], F32, tag=f"nwt_y_{tag}")
    t1 = pool.tile([P, n], F32, tag=f"nwt_t_{tag}")
    xh = pool.tile([P, n], F32, tag=f"nwt_xh_{tag}")
    # y0 bits = magic - (bits(x) >> 1)
    nc.vector.tensor_tensor(
        t1.bitcast(U32), x.bitcast(U32), shift1[:, 0:n], op=OP.logical_shift_right
    )
    nc.vector.tensor_tensor(
        y.bitcast(U32), magic[:, 0:n], t1.bitcast(U32), op=OP.subtract
    )
    nc.vector.tensor_scalar(xh, x, 0.5, None, op0=OP.mult)
    for _ in range(2):
        # y = y * (1.5 - xh * y * y)
        nc.vector.tensor_tensor(t1, y, y, op=OP.mult)
        nc.vector.tensor_tensor(t1, t1, xh, op=OP.mult)
        nc.vector.tensor_scalar(t1, t1, -1.0, 1.5, op0=OP.mult, op1=OP.add)
        nc.vector.tensor_tensor(y, y, t1, op=OP.mult)
    return y


def build_consts():
    """Host-side constant matrices shipped as extra kernel inputs."""
    # lhsT for c_t = L @ w_t  => upper-tri incl diag
    ut = np.triu(np.ones((P, P), np.float32))
    # sliding-window selector: column 31 is ones; E_t = slide[:, 31-t : 63-t]
    slide = np.zeros((P, 2 * T - 1), np.float32)
    slide[:, T - 1] = 1.0
    # lhsT for carries = Lx32 @ colsums => strictly upper tri [32, 32]
    sxt = np.triu(np.ones((T, T), np.float32), 1)
    # carry-broadcast selectors: esel[t] is [32, 128] with row t all ones,
    # so esel[t].T @ carries = ones(128) (x) carries[t]
    esel = np.zeros((T, T, P), np.float32)
    for t in range(T):
        esel[t, t, :] = 1.0
    esel = esel.transpose(1, 0, 2).reshape(T, T * P)
    magic = np.full((P, GRP * T), _MAGIC, np.uint32)
    shift1 = np.full((P, GRP * T), 1, np.uint32)
    return {
        "c_ut": ut, "c_slide": slide, "c_sxt": sxt, "c_esel": esel,
        "c_magic": magic, "c_shift1": shift1,
    }


def build_nc():
    nc = bacc.Bacc(
        "TRN2", target_bir_lowering=False, debug=False, num_devices=1
    )
    qd = nc.declare_dram_parameter("Q", [BHC, S, D], F32, isOutput=False)
    kd = nc.declare_dram_parameter("K", [BHC, S, D], F32, isOutput=False)
    vd = nc.declare_dram_parameter("V", [BHC, S, D], F32, isOutput=False)
    utd = nc.declare_dram_parameter("c_ut", [P, P], F32, isOutput=False)
    slided = nc.declare_dram_parameter("c_slide", [P, 2 * T - 1], F32, isOutput=False)
    sxtd = nc.declare_dram_parameter("c_sxt", [T, T], F32, isOutput=False)
    eseld = nc.declare_dram_parameter("c_esel", [T, T * P], F32, isOutput=False)
    magicd = nc.declare_dram_parameter("c_magic", [P, GRP * T], U32, isOutput=False)
    shiftd = nc.declare_dram_parameter("c_shift1", [P, GRP * T], U32, isOutput=False)
    outd = nc.declare_dram_parameter("out", [BHC, S, D], F32, isOutput=True)

    with tile.TileContext(nc) as tc:
        with (
            tc.tile_pool(name="consts", bufs=1) as consts,
            tc.tile_pool(name="vpool", bufs=GRP + 1) as vpool,
            tc.tile_pool(name="cpool", bufs=GRP) as cpool,
            tc.tile_pool(name="opool", bufs=GRP) as opool,
            tc.tile_pool(name="scr", bufs=2) as scr,
            tc.tile_pool(name="small", bufs=2) as small,
            tc.tile_pool(name="rowm", bufs=2) as rowm,
            tc.tile_pool(name="pc", bufs=4, space="PSUM") as ppc,
            tc.tile_pool(name="pcs", bufs=2, space="PSUM") as ppcs,
            tc.tile_pool(name="pcar", bufs=2, space="PSUM") as ppcar,
        ):
            ut = consts.tile([P, P], F32)
            nc.sync.dma_start(out=ut, in_=utd[:, :])
            slide = consts.tile([P, 2 * T - 1], F32)
            nc.sync.dma_start(out=slide, in_=slided[:, :])
            sxt = consts.tile([T, T], F32)
            nc.sync.dma_start(out=sxt, in_=sxtd[:, :])
            esel = consts.tile([T, T * P], F32)
            nc.sync.dma_start(out=esel, in_=eseld[:, :])
            magic = consts.tile([P, GRP * T], U32)
            nc.sync.dma_start(out=magic, in_=magicd[:, :])
            shift1 = consts.tile([P, GRP * T], U32)
            nc.sync.dma_start(out=shift1, in_=shiftd[:, :])

            for g in range(NG1):
                bhs = [g * GRP + i for i in range(GRP)]
                n = GRP * T
                # group stat tiles: [128, GRP*32]
                ccg = rowm.tile([P, n], F32, tag="ccg")
                qqg = rowm.tile([P, n], F32, tag="qqg")
                kkg = rowm.tile([P, n], F32, tag="kkg")

                vts, cts = {}, {}
                for i, bh in enumerate(bhs):
                    vt = vpool.tile([P, T, D], F32, tag="v")
                    nc.sync.dma_start(
                        out=vt, in_=vd[bh].rearrange("(t p) d -> p t d", p=P)
                    )
                    vts[bh] = vt
                    # silu(V) and w = silu(V)*V
                    sv = scr.tile([P, T, D], F32, tag="sv")
                    nc.scalar.activation(sv, vt, AF.Silu)
                    wt = scr.tile([P, T, D], F32, tag="w")
                    nc.vector.tensor_tensor(wt, sv, vt, op=OP.mult)

                    # Q/K norms^2 -> qqg/kkg
                    for name, dram, dst in (("q", qd, qqg), ("k", kd, kkg)):
                        xt = scr.tile([P, T, D], F32, tag="qk")
                        nc.sync.dma_start(
                            out=xt, in_=dram[bh].rearrange("(t p) d -> p t d", p=P)
                        )
                        sq = scr.tile([P, T, D], F32, tag="sq")
                        nc.gpsimd.tensor_tensor(sq, xt, xt, op=OP.mult)
                        nc.vector.tensor_reduce(
                            dst[:, i * T:(i + 1) * T], sq, axis=AX.X, op=OP.add
                        )

                    # colsums[t, :] = sum_p w_t[p, :]
                    pcs = ppcs.tile([T, D], F32)
                    for t in range(T):
                        nc.tensor.matmul(
                            pcs, slide[:, T - 1 - t:2 * T - 1 - t], wt[:, t, :],
                            start=(t == 0), stop=(t == T - 1),
                        )
                    cs_s = small.tile([T, D], F32, tag="cs")
                    nc.scalar.copy(cs_s, pcs)
                    # carries = exclusive prefix of colsums
                    pcar = ppcar.tile([T, D], F32)
                    nc.tensor.matmul(pcar, sxt, cs_s, start=True, stop=True)
                    car_s = small.tile([T, D], F32, tag="car")
                    nc.scalar.copy(car_s, pcar)

                    # c = L @ w + bcast(carries), 8 tiles per L-matmul
                    ct = cpool.tile([P, T, D], F32, tag="c")
                    cts[bh] = ct
                    CH = 8
                    for ci in range(T // CH):
                        pc = ppc.tile([P, CH, D], F32)
                        nc.tensor.matmul(
                            pc, ut, wt[:, ci * CH:(ci + 1) * CH, :],
                            start=True, stop=False,
                        )
                        for tt in range(CH):
                            t = ci * CH + tt
                            nc.tensor.matmul(
                                pc[:, tt, :], esel[:, t * P:(t + 1) * P], car_s,
                                start=False, stop=(tt == CH - 1),
                            )
                        nc.scalar.copy(ct[:, ci * CH:(ci + 1) * CH, :], pc)

                    # cc = sum_d c^2
                    sqc = scr.tile([P, T, D], F32, tag="sq")
                    nc.scalar.square(sqc, ct)
                    nc.vector.tensor_reduce(
                        ccg[:, i * T:(i + 1) * T], sqc, axis=AX.X, op=OP.add
                    )

                # --- group row math: s1 = rsqrt((cc/64+eps) * r^2) ---
                rq = _newton_rsqrt(nc, rowm, qqg, magic, shift1, n, "q")
                nc.vector.tensor_tensor(rq, rq, qqg, op=OP.mult)  # sqrt(qq)
                rk = _newton_rsqrt(nc, rowm, kkg, magic, shift1, n, "k")
                nc.vector.tensor_tensor(rk, rk, kkg, op=OP.mult)  # sqrt(kk)
                r = rowm.tile([P, n], F32, tag="r")
                nc.vector.scalar_tensor_tensor(
                    r, rq, 1.0, rk, op0=OP.add, op1=OP.add
                )
                ccp = rowm.tile([P, n], F32, tag="ccp")
                nc.vector.tensor_scalar(
                    ccp, ccg, 1.0 / D, RMS_EPS, op0=OP.mult, op1=OP.add
                )
                nc.vector.tensor_tensor(ccp, ccp, r, op=OP.mult)
                nc.vector.tensor_tensor(ccp, ccp, r, op=OP.mult)
                s1 = _newton_rsqrt(nc, rowm, ccp, magic, shift1, n, "s1")

                oog = rowm.tile([P, n], F32, tag="oog")
                o0s = {}
                for i, bh in enumerate(bhs):
                    ct, vt = cts[bh], vts[bh]
                    ms = scr.tile([P, T, D], F32, tag="ms")
                    for t in range(T):
                        nc.vector.tensor_scalar(
                            ms[:, t, :], ct[:, t, :],
                            s1[:, i * T + t:i * T + t + 1], None, op0=OP.mult,
                        )
                    gt = scr.tile([P, T, D], F32, tag="g")
                    nc.scalar.activation(gt, ms, AF.Silu)
                    o0 = opool.tile([P, T, D], F32, tag="o0")
                    nc.vector.scalar_tensor_tensor(
                        o0, gt, 1.0, vt, op0=OP.add, op1=OP.mult
                    )
                    o0s[bh] = o0
                    sqo = scr.tile([P, T, D], F32, tag="sq")
                    nc.scalar.square(sqo, o0)
                    nc.vector.tensor_reduce(
                        oog[:, i * T:(i + 1) * T], sqo, axis=AX.X, op=OP.add
                    )

                oop = rowm.tile([P, n], F32, tag="ccp")
                nc.vector.tensor_scalar(
                    oop, oog, 1.0 / D, RMS_EPS, op0=OP.mult, op1=OP.add
                )
                s2 = _newton_rsqrt(nc, rowm, oop, magic, shift1, n, "s2")

                for i, bh in enumerate(bhs):
                    o0 = o0s[bh]
                    ot = opool.tile([P, T, D], F32, tag="ot")
                    for t in range(T):
                        nc.vector.tensor_scalar(
                            ot[:, t, :], o0[:, t, :],
                            s2[:, i * T + t:i * T + t + 1], None, op0=OP.mult,
                        )
                    nc.sync.dma_start(
                        out=outd[bh].rearrange("(t p) d -> p t d", p=P), in_=ot
                    )
    nc.finalize()
    return nc


_NC_CACHE = None


def kernel(Q, K, V):
    global _NC_CACHE
    if _NC_CACHE is None:
        _NC_CACHE = build_nc()
    nc = _NC_CACHE
    consts = build_consts()
    Qs = np.ascontiguousarray(np.asarray(Q, np.float32).reshape(BH, S, D))
    Ks = np.ascontiguousarray(np.asarray(K, np.float32).reshape(BH, S, D))
    Vs = np.ascontiguousarray(np.asarray(V, np.float32).reshape(BH, S, D))
    in_maps = []
    for c in range(NCORES):
        sl = slice(c * BHC, (c + 1) * BHC)
        in_maps.append(
            {"Q": Qs[sl], "K": Ks[sl], "V": Vs[sl], **consts}
        )
    res = run_bass_kernel_spmd(nc, in_maps, list(range(NCORES)))
    out = np.concatenate([res.results[c]["out"] for c in range(NCORES)], axis=0)
    return out.reshape(B, H, S, D)


# revision 30
# speedup vs baseline: 493.7906x; 388.9045x over previous
"""EvoAttentionCausal Trainium2 kernel (8-core SPMD).

Computes, per (b, h):
    v_swiglu = silu(V) * V
    c        = cumsum(v_swiglu, axis=S)
    c_n      = c * rsqrt(mean(c^2, D) + 1e-5)
    r        = ||Q||_D + ||K||_D + 1          (L2 eps 1e-8 is negligible)
    mstate   = c_n / r
    out0     = V + silu(mstate) * V
    out      = out0 * rsqrt(mean(out0^2, D) + 1e-5)

Sharding: B*H = 64 (b,h) units, 8 per NeuronCore, fully independent.

Layout: pair-packed sequence on partitions: s = 256*t + 2*p + e
(t = 16 meta-tiles, p = 128 partitions, e in {0,1}), so each partition's
HBM run is 2 rows = 512 B -> full-rate DMA descriptors.

Cumsum over S via tensor-engine matmuls (contract over partitions):
    c0[p,t] = carry[t] + L@w0 + Lstrict@w1      (e=0 positions)
    c1[p,t] = carry[t] + L@w0 + L@w1            (e=1 positions)
    colsums[t] = E_t^T@w0 + E_t^T@w1 ; carries = exclusive-prefix (Lx16)
    carry broadcast via esel[t]^T @ carries (PSUM accumulate)
Per-(s) scalars applied with stride-0 broadcast APs in tensor_tensor.
All rsqrt/sqrt on DVE (Newton + bit trick) so ACT stays on the
silu_and_others table set (Silu/Square/Copy/Identity) -> one table load.
"""

import sys

sys.path.insert(0, "/opt/trn_rl_repo")

import numpy as np

import concourse.bass as bass
import concourse.bacc as bacc
import concourse.mybir as mybir
import concourse.tile as tile
from concourse.bass_utils import run_bass_kernel_spmd

F32 = mybir.dt.float32
U32 = mybir.dt.uint32
AF = mybir.ActivationFunctionType
OP = mybir.AluOpType
AX = mybir.AxisListType

B, H, S, D = 4, 16, 4096, 64
NCORES = 8
BH = B * H              # 64
BHC = BH // NCORES      # 8 bh units per core
P = 128                 # partitions
TT = S // (2 * P)       # 16 meta-tiles (256 seq positions each)
NSC = 2 * TT            # 32 per-partition scalars per bh
GRP = 2                 # bh per row-math group
NG = BHC // GRP
RMS_EPS = 1e-5
CH = 8                  # meta-tiles per batched L-matmul (N=512)

_MAGIC = 0x5F3759DF


def _newton_rsqrt(nc, pool, x, magic, shift1, n, tag):
    """rsqrt(x) elementwise on a [128, n] f32 SBUF tile via bit trick +
    2 Newton iterations. Returns a fresh tile."""
    y = pool.tile([P, n], F32, tag=f"nwt_y_{tag}")
    t1 = pool.tile([P, n], F32, tag=f"nwt_t_{tag}")
    xh = pool.tile([P, n], F32, tag=f"nwt_xh_{tag}")
    nc.vector.tensor_tensor(
        t1.bitcast(U32), x.bitcast(U32), shift1[:, 0:n], op=OP.logical_shift_right
    )
    nc.vector.tensor_tensor(
        y.bitcast(U32), magic[:, 0:n], t1.bitcast(U32), op=OP.subtract
    )
    nc.vector.tensor_scalar(xh, x, 0.5, None, op0=OP.mult)
    for _ in range(2):
        nc.vector.tensor_tensor(t1, y, y, op=OP.mult)
        nc.vector.tensor_tensor(t1, t1, xh, op=OP.mult)
        nc.vector.tensor_scalar(t1, t1, -1.0, 1.5, op0=OP.mult, op1=OP.add)
        nc.vector.tensor_tensor(y, y, t1, op=OP.mult)
    return y


def build_consts():
    """Host-side constant matrices shipped as extra kernel inputs."""
    ut = np.triu(np.ones((P, P), np.float32))        # (L incl diag).T
    uts = np.triu(np.ones((P, P), np.float32), 1)    # (L strict).T
    # sliding-window col selector: E_t = slide[:, TT-1-t : 2*TT-1-t]
    slide = np.zeros((P, 2 * TT - 1), np.float32)
    slide[:, TT - 1] = 1.0
    sxt = np.triu(np.ones((TT, TT), np.float32), 1)  # exclusive prefix lhsT
    # esel[t]: [TT, 128] with row t all ones
    esel = np.zeros((TT, TT, P), np.float32)
    for t in range(TT):
        esel[t, t, :] = 1.0
    esel = esel.transpose(1, 0, 2).reshape(TT, TT * P)
    magic = np.full((P, GRP * NSC), _MAGIC, np.uint32)
    shift1 = np.full((P, GRP * NSC), 1, np.uint32)
    return {
        "c_ut": ut, "c_uts": uts, "c_slide": slide, "c_sxt": sxt,
        "c_esel": esel, "c_magic": magic, "c_shift1": shift1,
    }


def _pp(dram_bh):
    """Pair-packed view of one bh [S, D] slice: s = 256 t + 2 p + e."""
    return dram_bh.rearrange("(t p e) d -> p t e d", p=P, e=2)


def build_nc():
    nc = bacc.Bacc(
        "TRN2", target_bir_lowering=False, debug=False, num_devices=1
    )
    qd = nc.declare_dram_parameter("Q", [BHC, S, D], F32, isOutput=False)
    kd = nc.declare_dram_parameter("K", [BHC, S, D], F32, isOutput=False)
    vd = nc.declare_dram_parameter("V", [BHC, S, D], F32, isOutput=False)
    utd = nc.declare_dram_parameter("c_ut", [P, P], F32, isOutput=False)
    utsd = nc.declare_dram_parameter("c_uts", [P, P], F32, isOutput=False)
    slided = nc.declare_dram_parameter("c_slide", [P, 2 * TT - 1], F32, isOutput=False)
    sxtd = nc.declare_dram_parameter("c_sxt", [TT, TT], F32, isOutput=False)
    eseld = nc.declare_dram_parameter("c_esel", [TT, TT * P], F32, isOutput=False)
    magicd = nc.declare_dram_parameter("c_magic", [P, GRP * NSC], U32, isOutput=False)
    shiftd = nc.declare_dram_parameter("c_shift1", [P, GRP * NSC], U32, isOutput=False)
    outd = nc.declare_dram_parameter("out", [BHC, S, D], F32, isOutput=True)

    with tile.TileContext(nc) as tc:
        with (
            tc.tile_pool(name="consts", bufs=1) as consts,
            tc.tile_pool(name="vpool", bufs=3) as vpool,
            tc.tile_pool(name="cpool", bufs=2) as cpool,
            tc.tile_pool(name="opool", bufs=2) as opool,
            tc.tile_pool(name="scr", bufs=2) as scr,
            tc.tile_pool(name="small", bufs=2) as small,
            tc.tile_pool(name="rowm", bufs=2) as rowm,
            tc.tile_pool(name="pc0", bufs=3, space="PSUM") as ppc0,
            tc.tile_pool(name="pc1", bufs=3, space="PSUM") as ppc1,
            tc.tile_pool(name="pcs", bufs=1, space="PSUM") as ppcs,
            tc.tile_pool(name="pcar", bufs=1, space="PSUM") as ppcar,
        ):
            ut = consts.tile([P, P], F32)
            nc.sync.dma_start(out=ut, in_=utd[:, :])
            uts = consts.tile([P, P], F32)
            nc.sync.dma_start(out=uts, in_=utsd[:, :])
            slide = consts.tile([P, 2 * TT - 1], F32)
            nc.sync.dma_start(out=slide, in_=slided[:, :])
            sxt = consts.tile([TT, TT], F32)
            nc.sync.dma_start(out=sxt, in_=sxtd[:, :])
            esel = consts.tile([TT, TT * P], F32)
            nc.sync.dma_start(out=esel, in_=eseld[:, :])
            magic = consts.tile([P, GRP * NSC], U32)
            nc.sync.dma_start(out=magic, in_=magicd[:, :])
            shift1 = consts.tile([P, GRP * NSC], U32)
            nc.sync.dma_start(out=shift1, in_=shiftd[:, :])

            for g in range(NG):
                bhs = [g * GRP + i for i in range(GRP)]
                n = GRP * NSC
                ccg = rowm.tile([P, n], F32, tag="ccg")
                qqg = rowm.tile([P, n], F32, tag="qqg")
                kkg = rowm.tile([P, n], F32, tag="kkg")

                vts, cts = {}, {}
                for i, bh in enumerate(bhs):
                    vt = vpool.tile([P, TT, 2, D], F32, tag="v")
                    nc.sync.dma_start(out=vt, in_=_pp(vd[bh]))
                    vts[bh] = vt
                    sv = scr.tile([P, TT, 2, D], F32, tag="sv")
                    nc.scalar.activation(sv, vt, AF.Silu)
                    wt = scr.tile([P, TT, 2, D], F32, tag="w")
                    nc.gpsimd.tensor_tensor(wt, sv, vt, op=OP.mult)

                    # Q/K norms^2 (per (t, e) row)
                    for dram, dst in ((qd, qqg), (kd, kkg)):
                        xt = scr.tile([P, TT, 2, D], F32, tag="qk")
                        nc.sync.dma_start(out=xt, in_=_pp(dram[bh]))
                        sq = scr.tile([P, TT, 2, D], F32, tag="sq")
                        nc.gpsimd.tensor_tensor(sq, xt, xt, op=OP.mult)
                        nc.vector.tensor_reduce(
                            dst[:, i * NSC:(i + 1) * NSC],
                            sq, axis=AX.X, op=OP.add,
                        )

                    w0 = wt[:, :, 0, :]
                    w1 = wt[:, :, 1, :]

                    # colsums[t] = sum_p (w0+w1)[p, t]
                    pcs = ppcs.tile([TT, D], F32)
                    for t in range(TT):
                        win = slide[:, TT - 1 - t:2 * TT - 1 - t]
                        nc.tensor.matmul(pcs, win, w0[:, t, :],
                                         start=(t == 0), stop=False)
                        nc.tensor.matmul(pcs, win, w1[:, t, :],
                                         start=False, stop=(t == TT - 1))
                    cs_s = small.tile([TT, D], F32, tag="cs")
                    nc.scalar.copy(cs_s, pcs)
                    pcar = ppcar.tile([TT, D], F32)
                    nc.tensor.matmul(pcar, sxt, cs_s, start=True, stop=True)
                    car_s = small.tile([TT, D], F32, tag="car")
                    nc.scalar.copy(car_s, pcar)

                    # c0/c1 chunks + carry broadcast
                    ct = cpool.tile([P, TT, 2, D], F32, tag="c")
                    cts[bh] = ct
                    for ci in range(TT // CH):
                        tsl = slice(ci * CH, (ci + 1) * CH)
                        pc0 = ppc0.tile([P, CH, D], F32)
                        pc1 = ppc1.tile([P, CH, D], F32)
                        nc.tensor.matmul(pc0, ut, w0[:, tsl, :],
                                         start=True, stop=False)
                        nc.tensor.matmul(pc0, uts, w1[:, tsl, :],
                                         start=False, stop=False)
                        nc.tensor.matmul(pc1, ut, w0[:, tsl, :],
                                         start=True, stop=False)
                        nc.tensor.matmul(pc1, ut, w1[:, tsl, :],
                                         start=False, stop=False)
                        for tt_ in range(CH):
                            t = ci * CH + tt_
                            es = esel[:, t * P:(t + 1) * P]
                            nc.tensor.matmul(pc0[:, tt_, :], es, car_s,
                                             start=False, stop=(tt_ == CH - 1))
                            nc.tensor.matmul(pc1[:, tt_, :], es, car_s,
                                             start=False, stop=(tt_ == CH - 1))
                        nc.scalar.copy(ct[:, tsl, 0, :], pc0)
                        nc.scalar.copy(ct[:, tsl, 1, :], pc1)

                    # cc = sum_d c^2 per (t, e)
                    sqc = scr.tile([P, TT, 2, D], F32, tag="sq")
                    nc.scalar.square(sqc, ct)
                    nc.vector.tensor_reduce(
                        ccg[:, i * NSC:(i + 1) * NSC],
                        sqc, axis=AX.X, op=OP.add,
                    )

                # --- group row math: s1 = rsqrt((cc/64+eps) * r^2) ---
                rq = _newton_rsqrt(nc, rowm, qqg, magic, shift1, n, "q")
                nc.vector.tensor_tensor(rq, rq, qqg, op=OP.mult)   # sqrt(qq)
                rk = _newton_rsqrt(nc, rowm, kkg, magic, shift1, n, "k")
                nc.vector.tensor_tensor(rk, rk, kkg, op=OP.mult)   # sqrt(kk)
                r = rowm.tile([P, n], F32, tag="r")
                nc.vector.scalar_tensor_tensor(
                    r, rq, 1.0, rk, op0=OP.add, op1=OP.add
                )
                ccp = rowm.tile([P, n], F32, tag="ccp")
                nc.vector.tensor_scalar(
                    ccp, ccg, 1.0 / D, RMS_EPS, op0=OP.mult, op1=OP.add
                )
                nc.vector.tensor_tensor(ccp, ccp, r, op=OP.mult)
                nc.vector.tensor_tensor(ccp, ccp, r, op=OP.mult)
                s1 = _newton_rsqrt(nc, rowm, ccp, magic, shift1, n, "s1")

                oog = rowm.tile([P, n], F32, tag="oog")
                o0s = {}
                for i, bh in enumerate(bhs):
                    ct, vt = cts[bh], vts[bh]
                    s1b = s1[:, i * NSC:(i + 1) * NSC].rearrange(
                        "p (t e) -> p t e", e=2)[:, :, :, None].broadcast_to(
                        [P, TT, 2, D])
                    ms = scr.tile([P, TT, 2, D], F32, tag="ms")
                    nc.gpsimd.tensor_tensor(ms, ct, s1b, op=OP.mult)
                    gt = scr.tile([P, TT, 2, D], F32, tag="g")
                    nc.scalar.activation(gt, ms, AF.Silu)
                    o0 = opool.tile([P, TT, 2, D], F32, tag="o0")
                    nc.vector.scalar_tensor_tensor(
                        o0, gt, 1.0, vt, op0=OP.add, op1=OP.mult
                    )
                    o0s[bh] = o0
                    sqo = scr.tile([P, TT, 2, D], F32, tag="sq")
                    nc.gpsimd.tensor_tensor(sqo, o0, o0, op=OP.mult)
                    nc.vector.tensor_reduce(
                        oog[:, i * NSC:(i + 1) * NSC],
                        sqo, axis=AX.X, op=OP.add,
                    )

                oop = rowm.tile([P, n], F32, tag="ccp")
                nc.vector.tensor_scalar(
                    oop, oog, 1.0 / D, RMS_EPS, op0=OP.mult, op1=OP.add
                )
                s2 = _newton_rsqrt(nc, rowm, oop, magic, shift1, n, "s2")

                for i, bh in enumerate(bhs):
                    o0 = o0s[bh]
                    s2b = s2[:, i * NSC:(i + 1) * NSC].rearrange(
                        "p (t e) -> p t e", e=2)[:, :, :, None].broadcast_to(
                        [P, TT, 2, D])
                    ot = opool.tile([P, TT, 2, D], F32, tag="ot")
                    nc.gpsimd.tensor_tensor(ot, o0, s2b, op=OP.mult)
                    nc.sync.dma_start(out=_pp(outd[bh]), in_=ot)
    nc.finalize()
    return nc


_NC_CACHE = None


def kernel(Q, K, V):
    global _NC_CACHE
    if _NC_CACHE is None:
        _NC_CACHE = build_nc()
    nc = _NC_CACHE
    consts = build_consts()
    Qs = np.ascontiguousarray(np.asarray(Q, np.float32).reshape(BH, S, D))
    Ks = np.ascontiguousarray(np.asarray(K, np.float32).reshape(BH, S, D))
    Vs = np.ascontiguousarray(np.asarray(V, np.float32).reshape(BH, S, D))
    in_maps = []
    for c in range(NCORES):
        sl = slice(c * BHC, (c + 1) * BHC)
        in_maps.append({"Q": Qs[sl], "K": Ks[sl], "V": Vs[sl], **consts})
    res = run_bass_kernel_spmd(nc, in_maps, list(range(NCORES)))
    out = np.concatenate([res.results[c]["out"] for c in range(NCORES)], axis=0)
    return out.reshape(B, H, S, D)


# revision 38
# speedup vs baseline: 615.8463x; 1.2472x over previous
"""EvoAttentionCausal Trainium2 kernel (8-core SPMD).

Computes, per (b, h):
    v_swiglu = silu(V) * V
    c        = cumsum(v_swiglu, axis=S)
    c_n      = c * rsqrt(mean(c^2, D) + 1e-5)
    r        = ||Q||_D + ||K||_D + 1          (L2 eps 1e-8 is negligible)
    mstate   = c_n / r
    out0     = V + silu(mstate) * V
    out      = out0 * rsqrt(mean(out0^2, D) + 1e-5)

Sharding: B*H = 64 (b,h) units, 8 per NeuronCore, fully independent.

Layout: pair-packed sequence on partitions: s = 256*t + 2*p + e
(t = 16 meta-tiles, p = 128 partitions, e in {0,1}), so each partition's
HBM run is 2 rows = 512 B -> full-rate DMA descriptors.

Cumsum over S via tensor-engine matmuls (contract over partitions):
    c0[p,t] = carry[t] + L@w0 + Lstrict@w1      (e=0 positions)
    c1[p,t] = carry[t] + L@w0 + L@w1            (e=1 positions)
    colsums[t] = E_t^T@w0 + E_t^T@w1 ; carries = exclusive-prefix (Lx16)
    carry broadcast via esel[t]^T @ carries (PSUM accumulate)
Per-(s) scalars applied with stride-0 broadcast APs in tensor_tensor.
All rsqrt/sqrt on DVE (Newton + bit trick) so ACT stays on the
silu_and_others table set (Silu/Square/Copy/Identity) -> one table load.
"""

import sys

sys.path.insert(0, "/opt/trn_rl_repo")

import numpy as np

import concourse.bass as bass
import concourse.bacc as bacc
import concourse.mybir as mybir
import concourse.tile as tile
from concourse.bass_utils import run_bass_kernel_spmd

F32 = mybir.dt.float32
U32 = mybir.dt.uint32
AF = mybir.ActivationFunctionType
OP = mybir.AluOpType
AX = mybir.AxisListType

B, H, S, D = 4, 16, 4096, 64
NCORES = 8
BH = B * H              # 64
BHC = BH // NCORES      # 8 bh units per core
P = 128                 # partitions
TT = S // (2 * P)       # 16 meta-tiles (256 seq positions each)
NSC = 2 * TT            # 32 per-partition scalars per bh
GRP = 2                 # bh per row-math group
NG = BHC // GRP
RMS_EPS = 1e-5
CH = 8                  # meta-tiles per batched L-matmul (N=512)

_MAGIC = 0x5F3759DF


def _newton_rsqrt(nc, pool, x, magic, shift1, n, tag):
    """rsqrt(x) elementwise on a [128, n] f32 SBUF tile via bit trick +
    2 Newton iterations. Returns a fresh tile."""
    y = pool.tile([P, n], F32, tag=f"nwt_y_{tag}")
    t1 = pool.tile([P, n], F32, tag=f"nwt_t_{tag}")
    nc.vector.tensor_tensor(
        t1.bitcast(U32), x.bitcast(U32), shift1[:, 0:n], op=OP.logical_shift_right
    )
    nc.vector.tensor_tensor(
        y.bitcast(U32), magic[:, 0:n], t1.bitcast(U32), op=OP.subtract
    )
    # two Newton iterations, fused via scalar_tensor_tensor; the second
    # iteration cancels the sign flip of the first:
    #   y' = (0.5*x*y^2 - 1.5) * y  ==  -y_newton
    for _ in range(2):
        nc.vector.tensor_tensor(t1, y, y, op=OP.mult)
        nc.vector.scalar_tensor_tensor(t1, t1, 0.5, x, op0=OP.mult, op1=OP.mult)
        nc.vector.scalar_tensor_tensor(y, t1, 1.5, y, op0=OP.subtract, op1=OP.mult)
    return y


def build_consts():
    """Host-side constant matrices shipped as extra kernel inputs."""
    ut = np.triu(np.ones((P, P), np.float32))        # (L incl diag).T
    uts = np.triu(np.ones((P, P), np.float32), 1)    # (L strict).T
    # sliding-window col selector: E_t = slide[:, TT-1-t : 2*TT-1-t]
    slide = np.zeros((P, 2 * TT - 1), np.float32)
    slide[:, TT - 1] = 1.0
    sxt = np.triu(np.ones((TT, TT), np.float32), 1)  # exclusive prefix lhsT
    # esel[t]: [TT, 128] with row t all ones
    esel = np.zeros((TT, TT, P), np.float32)
    for t in range(TT):
        esel[t, t, :] = 1.0
    esel = esel.transpose(1, 0, 2).reshape(TT, TT * P)
    magic = np.full((P, GRP * NSC), _MAGIC, np.uint32)
    shift1 = np.full((P, GRP * NSC), 1, np.uint32)
    return {
        "c_ut": ut, "c_uts": uts, "c_slide": slide, "c_sxt": sxt,
        "c_esel": esel, "c_magic": magic, "c_shift1": shift1,
    }


def _pp(dram_bh):
    """Pair-packed view of one bh [S, D] slice: s = 256 t + 2 p + e."""
    return dram_bh.rearrange("(t p e) d -> p t e d", p=P, e=2)


def build_nc():
    nc = bacc.Bacc(
        "TRN2", target_bir_lowering=False, debug=False, num_devices=1
    )
    qd = nc.declare_dram_parameter("Q", [BHC, S, D], F32, isOutput=False)
    kd = nc.declare_dram_parameter("K", [BHC, S, D], F32, isOutput=False)
    vd = nc.declare_dram_parameter("V", [BHC, S, D], F32, isOutput=False)
    utd = nc.declare_dram_parameter("c_ut", [P, P], F32, isOutput=False)
    utsd = nc.declare_dram_parameter("c_uts", [P, P], F32, isOutput=False)
    slided = nc.declare_dram_parameter("c_slide", [P, 2 * TT - 1], F32, isOutput=False)
    sxtd = nc.declare_dram_parameter("c_sxt", [TT, TT], F32, isOutput=False)
    eseld = nc.declare_dram_parameter("c_esel", [TT, TT * P], F32, isOutput=False)
    magicd = nc.declare_dram_parameter("c_magic", [P, GRP * NSC], U32, isOutput=False)
    shiftd = nc.declare_dram_parameter("c_shift1", [P, GRP * NSC], U32, isOutput=False)
    outd = nc.declare_dram_parameter("out", [BHC, S, D], F32, isOutput=True)

    with tile.TileContext(nc) as tc:
        with (
            tc.tile_pool(name="consts", bufs=1) as consts,
            tc.tile_pool(name="vpool", bufs=3) as vpool,
            tc.tile_pool(name="cpool", bufs=2) as cpool,
            tc.tile_pool(name="opool", bufs=2) as opool,
            tc.tile_pool(name="scr", bufs=2) as scr,
            tc.tile_pool(name="small", bufs=2) as small,
            tc.tile_pool(name="rowm", bufs=2) as rowm,
            tc.tile_pool(name="pc0", bufs=3, space="PSUM") as ppc0,
            tc.tile_pool(name="pc1", bufs=3, space="PSUM") as ppc1,
            tc.tile_pool(name="pcs", bufs=1, space="PSUM") as ppcs,
            tc.tile_pool(name="pcar", bufs=1, space="PSUM") as ppcar,
        ):
            ut = consts.tile([P, P], F32)
            nc.sync.dma_start(out=ut, in_=utd[:, :])
            uts = consts.tile([P, P], F32)
            nc.sync.dma_start(out=uts, in_=utsd[:, :])
            slide = consts.tile([P, 2 * TT - 1], F32)
            nc.sync.dma_start(out=slide, in_=slided[:, :])
            sxt = consts.tile([TT, TT], F32)
            nc.sync.dma_start(out=sxt, in_=sxtd[:, :])
            esel = consts.tile([TT, TT * P], F32)
            nc.sync.dma_start(out=esel, in_=eseld[:, :])
            magic = consts.tile([P, GRP * NSC], U32)
            nc.sync.dma_start(out=magic, in_=magicd[:, :])
            shift1 = consts.tile([P, GRP * NSC], U32)
            nc.sync.dma_start(out=shift1, in_=shiftd[:, :])

            for g in range(NG):
                bhs = [g * GRP + i for i in range(GRP)]
                n = GRP * NSC
                ccg = rowm.tile([P, n], F32, tag="ccg")
                qqg = rowm.tile([P, n], F32, tag="qqg")
                kkg = rowm.tile([P, n], F32, tag="kkg")

                vts, cts = {}, {}
                for i, bh in enumerate(bhs):
                    vt = vpool.tile([P, TT, 2, D], F32, tag="v")
                    nc.sync.dma_start(out=vt, in_=_pp(vd[bh]))
                    vts[bh] = vt
                    sv = scr.tile([P, TT, 2, D], F32, tag="sv")
                    wt = scr.tile([P, TT, 2, D], F32, tag="w")
                    for ci in range(2):
                        tsl = slice(ci * CH, (ci + 1) * CH)
                        nc.scalar.activation(sv[:, tsl], vt[:, tsl], AF.Silu)
                        nc.gpsimd.tensor_tensor(
                            wt[:, tsl], sv[:, tsl], vt[:, tsl], op=OP.mult)

                    # Q/K norms^2 (per (t, e) row)
                    for dram, dst in ((qd, qqg), (kd, kkg)):
                        xt = scr.tile([P, TT, 2, D], F32, tag="qk")
                        nc.sync.dma_start(out=xt, in_=_pp(dram[bh]))
                        sq = scr.tile([P, TT, 2, D], F32, tag="sq")
                        nc.gpsimd.tensor_tensor(sq, xt, xt, op=OP.mult)
                        nc.vector.tensor_reduce(
                            dst[:, i * NSC:(i + 1) * NSC],
                            sq, axis=AX.X, op=OP.add,
                        )

                    w1 = wt[:, :, 1, :]
                    # u = w0 + w1 (pair sums)
                    ux = scr.tile([P, TT, D], F32, tag="u")
                    nc.gpsimd.tensor_tensor(
                        ux, wt[:, :, 0, :], w1, op=OP.add)

                    # colsums[t] = sum_p u[p, t]
                    pcs = ppcs.tile([TT, D], F32)
                    for t in range(TT):
                        win = slide[:, TT - 1 - t:2 * TT - 1 - t]
                        nc.tensor.matmul(pcs, win, ux[:, t, :],
                                         start=(t == 0), stop=(t == TT - 1))
                    cs_s = small.tile([TT, D], F32, tag="cs")
                    nc.scalar.copy(cs_s, pcs)
                    pcar = ppcar.tile([TT, D], F32)
                    nc.tensor.matmul(pcar, sxt, cs_s, start=True, stop=True)
                    car_s = small.tile([TT, D], F32, tag="car")
                    nc.scalar.copy(car_s, pcar)

                    # c1 = carry + L@u per chunk; c0 = c1 - w1 elementwise
                    ct = cpool.tile([P, TT, 2, D], F32, tag="c")
                    cts[bh] = ct
                    for ci in range(TT // CH):
                        tsl = slice(ci * CH, (ci + 1) * CH)
                        pc1 = ppc1.tile([P, CH, D], F32)
                        nc.tensor.matmul(pc1, ut, ux[:, tsl, :],
                                         start=True, stop=False)
                        for tt_ in range(CH):
                            t = ci * CH + tt_
                            es = esel[:, t * P:(t + 1) * P]
                            nc.tensor.matmul(pc1[:, tt_, :], es, car_s,
                                             start=False, stop=(tt_ == CH - 1))
                        nc.scalar.copy(ct[:, tsl, 1, :], pc1)
                        nc.gpsimd.tensor_tensor(
                            ct[:, tsl, 0, :], ct[:, tsl, 1, :], w1[:, tsl, :],
                            op=OP.subtract)

                    # cc = sum_d c^2 per (t, e)
                    sqc = scr.tile([P, TT, 2, D], F32, tag="sq")
                    for ci in range(2):
                        tsl = slice(ci * CH, (ci + 1) * CH)
                        nc.scalar.square(sqc[:, tsl], ct[:, tsl])
                        nc.vector.tensor_reduce(
                            ccg[:, i * NSC + ci * CH * 2:
                                i * NSC + (ci + 1) * CH * 2],
                            sqc[:, tsl], axis=AX.X, op=OP.add,
                        )

                # --- group row math: s1 = rsqrt((cc/64+eps) * r^2) ---
                rq = _newton_rsqrt(nc, rowm, qqg, magic, shift1, n, "q")
                nc.vector.tensor_tensor(rq, rq, qqg, op=OP.mult)   # sqrt(qq)
                rk = _newton_rsqrt(nc, rowm, kkg, magic, shift1, n, "k")
                nc.vector.tensor_tensor(rk, rk, kkg, op=OP.mult)   # sqrt(kk)
                r = rowm.tile([P, n], F32, tag="r")
                nc.vector.scalar_tensor_tensor(
                    r, rq, 1.0, rk, op0=OP.add, op1=OP.add
                )
                ccp = rowm.tile([P, n], F32, tag="ccp")
                nc.vector.tensor_scalar(
                    ccp, ccg, 1.0 / D, RMS_EPS, op0=OP.mult, op1=OP.add
                )
                nc.vector.tensor_tensor(ccp, ccp, r, op=OP.mult)
                nc.vector.tensor_tensor(ccp, ccp, r, op=OP.mult)
                s1 = _newton_rsqrt(nc, rowm, ccp, magic, shift1, n, "s1")

                oog = rowm.tile([P, n], F32, tag="oog")
                o0s = {}
                for i, bh in enumerate(bhs):
                    ct, vt = cts[bh], vts[bh]
                    s1b = s1[:, i * NSC:(i + 1) * NSC].rearrange(
                        "p (t e) -> p t e", e=2)[:, :, :, None].broadcast_to(
                        [P, TT, 2, D])
                    ms = scr.tile([P, TT, 2, D], F32, tag="ms")
                    gt = scr.tile([P, TT, 2, D], F32, tag="g")
                    for ci in range(2):
                        tsl = slice(ci * CH, (ci + 1) * CH)
                        nc.gpsimd.tensor_tensor(
                            ms[:, tsl], ct[:, tsl], s1b[:, tsl], op=OP.mult)
                        nc.scalar.activation(gt[:, tsl], ms[:, tsl], AF.Silu)
                    o0 = opool.tile([P, TT, 2, D], F32, tag="o0")
                    for ci in range(2):
                        tsl = slice(ci * CH, (ci + 1) * CH)
                        nc.vector.scalar_tensor_tensor(
                            o0[:, tsl], gt[:, tsl], 1.0, vt[:, tsl],
                            op0=OP.add, op1=OP.mult
                        )
                    o0s[bh] = o0
                    sqo = scr.tile([P, TT, 2, D], F32, tag="sq")
                    for ci in range(2):
                        tsl = slice(ci * CH, (ci + 1) * CH)
                        nc.scalar.square(sqo[:, tsl], o0[:, tsl])
                        nc.vector.tensor_reduce(
                            oog[:, i * NSC + ci * CH * 2:
                                i * NSC + (ci + 1) * CH * 2],
                            sqo[:, tsl], axis=AX.X, op=OP.add,
                        )

                oop = rowm.tile([P, n], F32, tag="ccp")
                nc.vector.tensor_scalar(
                    oop, oog, 1.0 / D, RMS_EPS, op0=OP.mult, op1=OP.add
                )
                s2 = _newton_rsqrt(nc, rowm, oop, magic, shift1, n, "s2")

                for i, bh in enumerate(bhs):
                    o0 = o0s[bh]
                    s2b = s2[:, i * NSC:(i + 1) * NSC].rearrange(
                        "p (t e) -> p t e", e=2)[:, :, :, None].broadcast_to(
                        [P, TT, 2, D])
                    ot = opool.tile([P, TT, 2, D], F32, tag="ot")
                    op_ = _pp(outd[bh])
                    for ci in range(2):
                        tsl = slice(ci * CH, (ci + 1) * CH)
                        nc.gpsimd.tensor_tensor(
                            ot[:, tsl], o0[:, tsl], s2b[:, tsl], op=OP.mult)
                        nc.sync.dma_start(out=op_[:, tsl], in_=ot[:, tsl])
    nc.finalize()
    return nc


_NC_CACHE = None


def kernel(Q, K, V):
    global _NC_CACHE
    if _NC_CACHE is None:
        _NC_CACHE = build_nc()
    nc = _NC_CACHE
    consts = build_consts()
    Qs = np.ascontiguousarray(np.asarray(Q, np.float32).reshape(BH, S, D))
    Ks = np.ascontiguousarray(np.asarray(K, np.float32).reshape(BH, S, D))
    Vs = np.ascontiguousarray(np.asarray(V, np.float32).reshape(BH, S, D))
    in_maps = []
    for c in range(NCORES):
        sl = slice(c * BHC, (c + 1) * BHC)
        in_maps.append({"Q": Qs[sl], "K": Ks[sl], "V": Vs[sl], **consts})
    res = run_bass_kernel_spmd(nc, in_maps, list(range(NCORES)))
    out = np.concatenate([res.results[c]["out"] for c in range(NCORES)], axis=0)
    return out.reshape(B, H, S, D)
